# revision 1
# baseline (speedup 1.0000x reference)
"""Trainium2 Bass kernel for per-channel convolutional attention.

Reference computation (per batch b):
  q = wq @ x + bq ; k = wk @ x + bk ; v = wv @ x + bv     (1x1 convs, [128,256] weights)
  score[c,i,j] = sum_w q[c,i,w] k[c,j,w] / sqrt(128)
  attn = softmax(score, axis=j) ;  out[c,i,w] = sum_j attn[c,i,j] v[c,j,w]

Device algorithm (per core, batch-parallel over 8 cores, 2 batches/core):
  - Projection (fp32r matmuls, N=384): per image row h, x[:, :, h, :] as lhsT
    [ci, w] against wqkvT [ci, 384] -> psum [w, 384] -> evict to qkvT [w, 384, h]
    (bias-free: bk cancels in softmax; bq enters via K0sum trick; bv added at the end).
  - Attention per channel c (f16 matmuls):
      scoreT[j,i] = kT_c.T @ qT_c    (K=w)          [+ K0sum column via ones rhs]
      expT = exp(scoreT/s + (bq_c/s)*K0sum_j)        (bias AP per partition j)
      v_c  = PE-transpose(vT_c), with a ones column -> v1 [j, 129]
      out' = expT.T @ v1   -> [i, 128 out | denom]   (K=j)
      out  = out'[:, :128] * (1/denom) + bv_c        (per-partition scale at evict)
  - Softmax needs no max-subtraction: logits are in [-7, 9] for this model family
    (checked on host; exp stays in fp32 range).
Output written as [b, h, c, w] in DRAM (4KB DMA descriptors), host transposes.
"""
import math
import numpy as np
from contextlib import ExitStack

import concourse.bass as bass
import concourse.tile as tile
import concourse.mybir as mybir
from concourse import bacc, bass_utils
from concourse.masks import make_identity

F32 = mybir.dt.float32
F32R = mybir.dt.float32r
F16 = mybir.dt.float16
AF = mybir.ActivationFunctionType
ALU = mybir.AluOpType

B_LOCAL = 2          # batches per core
CIN = 256
C = 128              # q/k/v channels
H = 128
W = 128
QKV = 3 * C          # 384
HB = 16              # h rows per x DMA chunk
SCALE = 1.0 / math.sqrt(128.0)


def build(attn_dt=F16, proj_dt=F32R):
    nc = bacc.Bacc(trn_type="TRN2", debug=False)
    x_d = nc.dram_tensor("x", [B_LOCAL, CIN, H, W], proj_dt, kind="ExternalInput").ap()
    w_d = nc.dram_tensor("w", [2, 128, QKV], proj_dt, kind="ExternalInput").ap()
    bqs_d = nc.dram_tensor("bqs", [128], F32, kind="ExternalInput").ap()
    bvs_d = nc.dram_tensor("bvs", [128], F32, kind="ExternalInput").ap()
    # out layout: [b, h, c, w]  (c,w contiguous -> 4KB descriptors per partition)
    o_d = nc.dram_tensor("o", [B_LOCAL, H, C, W], F32, kind="ExternalOutput").ap()

    with ExitStack() as ctx:
        tc = ctx.enter_context(tile.TileContext(nc))
        singles = ctx.enter_context(tc.tile_pool(name="singles", bufs=1))
        xpool = ctx.enter_context(tc.tile_pool(name="xp", bufs=2))
        exp_pool = ctx.enter_context(tc.tile_pool(name="expp", bufs=4))
        small = ctx.enter_context(tc.tile_pool(name="small", bufs=6))
        out8_pool = ctx.enter_context(tc.tile_pool(name="out8", bufs=2))
        ps_proj = ctx.enter_context(tc.tile_pool(name="psproj", bufs=2, space="PSUM"))
        ps_score = ctx.enter_context(tc.tile_pool(name="psscore", bufs=2, space="PSUM"))
        ps_vt = ctx.enter_context(tc.tile_pool(name="psvt", bufs=2, space="PSUM"))
        ps_out = ctx.enter_context(tc.tile_pool(name="psout", bufs=2, space="PSUM"))

        w_sb = singles.tile([128, 2, QKV], proj_dt)
        nc.sync.dma_start(out=w_sb, in_=w_d.rearrange("t p c -> p t c"))
        bqs_sb = singles.tile([128, 128], F32)
        nc.gpsimd.dma_start(
            out=bqs_sb,
            in_=bass.AP(tensor=bqs_d.tensor, offset=bqs_d.offset,
                        ap=[[0, 128], [1, 128]]),
        )
        bvs_sb = singles.tile([128, 128], F32)
        nc.gpsimd.dma_start(
            out=bvs_sb,
            in_=bass.AP(tensor=bvs_d.tensor, offset=bvs_d.offset,
                        ap=[[0, 128], [1, 128]]),
        )
        ones_sb = singles.tile([128, 1], attn_dt)
        nc.vector.memset(ones_sb, 1.0)
        ident = singles.tile([128, 128], attn_dt)
        make_identity(nc, ident)
        # qkvT[w, ch, h]: ch in [0,128)=q, [128,256)=k, [256,384)=v
        qkvT = singles.tile([128, QKV, H], attn_dt)
        # v1 slots: 8 channel-pairs in flight; ones column preset once
        NSLOT = 8
        v1_all = singles.tile([128, 2 * NSLOT, 132], attn_dt)
        nc.vector.memset(v1_all[:, :, 128], 1.0)

        for b in range(B_LOCAL):
            # ---------------- projection ----------------
            for hb in range(0, H, HB):
                x_t = xpool.tile([128, 2, HB, W], proj_dt)
                nc.sync.dma_start(
                    out=x_t,
                    in_=x_d[b, :, hb:hb + HB, :].rearrange(
                        "(t p) h w -> p t h w", p=128),
                )
                for hh in range(HB):
                    h = hb + hh
                    pp = ps_proj.tile([128, 512], F32)
                    nc.tensor.matmul(pp[:, 0:QKV], x_t[:, 0, hh, :], w_sb[:, 0, :],
                                     start=True, stop=False)
                    nc.tensor.matmul(pp[:, 0:QKV], x_t[:, 1, hh, :], w_sb[:, 1, :],
                                     start=False, stop=True)
                    if h % 2 == 0:
                        nc.scalar.copy(qkvT[:, :, h], pp[:, 0:QKV])
                    else:
                        nc.vector.tensor_copy(qkvT[:, :, h], pp[:, 0:QKV])

            # ---------------- attention ----------------
            for p in range(C // 2):
                c0 = 2 * p
                slot = p % NSLOT
                ss = ps_score.tile([128, 2, 132], F32)
                for cc in range(2):
                    c = c0 + cc
                    kT = qkvT[:, C + c, :]
                    nc.tensor.matmul(ss[:, cc, 128:129], kT, ones_sb,
                                     start=True, stop=True)
                    nc.tensor.matmul(ss[:, cc, 0:128], kT, qkvT[:, c, :],
                                     start=True, stop=True)
                bias2 = small.tile([128, 2], F32, tag="bias2")
                nc.vector.tensor_mul(bias2, ss[:, :, 128], bqs_sb[:, c0:c0 + 2])
                ex = exp_pool.tile([128, 2, 128], attn_dt)
                vt = ps_vt.tile([128, 2, 132], attn_dt)
                for cc in range(2):
                    c = c0 + cc
                    nc.scalar.activation(ex[:, cc, :], ss[:, cc, 0:128], AF.Exp,
                                         bias=bias2[:, cc:cc + 1], scale=SCALE)
                    nc.tensor.transpose(vt[:, cc, 0:128], qkvT[:, 2 * C + c, :],
                                        ident)
                nc.vector.tensor_copy(v1_all[:, 2 * slot:2 * slot + 2, 0:128],
                                      vt[:, :, 0:128])
                po = ps_out.tile([128, 2, 132], F32)
                for cc in range(2):
                    nc.tensor.matmul(po[:, cc, 0:129], ex[:, cc, :],
                                     v1_all[:, 2 * slot + cc, 0:129],
                                     start=True, stop=True)
                recip2 = small.tile([128, 2], F32, tag="recip2")
                nc.vector.reciprocal(recip2, po[:, :, 128])
                if p % 4 == 0:
                    o8 = out8_pool.tile([128, 8, 128], F32)
                for cc in range(2):
                    c = c0 + cc
                    if cc == 0:
                        nc.vector.tensor_scalar(
                            out=o8[:, c % 8, :], in0=po[:, cc, 0:128],
                            scalar1=recip2[:, cc:cc + 1], scalar2=bvs_sb[:, c:c + 1],
                            op0=ALU.mult, op1=ALU.add)
                    else:
                        nc.scalar.activation(
                            o8[:, c % 8, :], po[:, cc, 0:128], AF.Identity,
                            bias=bvs_sb[:, c:c + 1], scale=recip2[:, cc:cc + 1])
                if p % 4 == 3:
                    nc.sync.dma_start(out=o_d[b, :, c0 - 6:c0 + 2, :], in_=o8)
    nc.finalize()
    return nc


_CACHE = {}


def _get_nc(attn_dt, proj_dt):
    key = (attn_dt, proj_dt)
    if key not in _CACHE:
        _CACHE[key] = build(attn_dt=attn_dt, proj_dt=proj_dt)
    return _CACHE[key]


def kernel(x, wq, bq, wk, bk, wv, bv, _trace=False):
    x = np.ascontiguousarray(np.asarray(x, dtype=np.float32))
    wq = np.asarray(wq, dtype=np.float32)
    wk = np.asarray(wk, dtype=np.float32)
    wv = np.asarray(wv, dtype=np.float32)
    bq = np.asarray(bq, dtype=np.float32)
    bv = np.asarray(bv, dtype=np.float32)

    wqkv = np.concatenate([wq, wk, wv], axis=0)          # [384, 256]
    wT = np.ascontiguousarray(wqkv.T).reshape(2, 128, QKV)  # [t, ci_p, 384]
    bqs = (bq * np.float32(SCALE)).astype(np.float32)

    nc = _get_nc(F16, F32R)
    in_maps = [
        {"x": x[2 * i:2 * i + 2], "w": wT, "bqs": bqs, "bvs": bv}
        for i in range(8)
    ]
    res = bass_utils.run_bass_kernel_spmd(
        nc, in_maps, core_ids=list(range(8)), trace=_trace)
    outs = []
    for i in range(8):
        o = res.results[i]["o"]                      # [2, h, c, w]
        outs.append(np.transpose(o, (0, 2, 1, 3)))   # [2, c, h, w]
    full = np.concatenate(outs, axis=0).astype(np.float32)
    if _trace:
        kernel._last_result = res
    return full



# revision 2
# speedup vs baseline: 1.0208x; 1.0208x over previous
"""Trainium2 Bass kernel for per-channel convolutional attention.

Reference computation (per batch b):
  q = wq @ x + bq ; k = wk @ x + bk ; v = wv @ x + bv     (1x1 convs, [128,256] weights)
  score[c,i,j] = sum_w q[c,i,w] k[c,j,w] / sqrt(128)
  attn = softmax(score, axis=j) ;  out[c,i,w] = sum_j attn[c,i,j] v[c,j,w]

Device algorithm (per core, batch-parallel over 8 cores, 2 batches/core),
f16 end-to-end (f32 PSUM accumulate; bytes on the wire halved vs f32):
  - Projection (f16 matmuls, N=384): per image row h, x[:, :, h, :] as lhsT
    [ci, w] against wqkvT [ci, 384] -> psum [w, 384] -> evict to qkvT [w, 384, h]
    (bias-free: bk cancels in softmax; bq enters via K0sum trick; bv added at the end).
  - Attention per channel c (f16 matmuls):
      scoreT[j,i] = kT_c.T @ qT_c    (K=w)          [+ K0sum column via ones rhs]
      expT = exp(scoreT/s + (bq_c/s)*K0sum_j)        (bias AP per partition j)
      v_c  = PE-transpose(vT_c), with a ones column -> v1 [j, 129]
      out' = expT.T @ v1   -> [i, 128 out | denom]   (K=j)
      out  = out'[:, :128] * (1/denom) + bv_c        (per-partition scale at evict)
  - Softmax needs no max-subtraction: logits are in [-7, 9] for this model family
    (checked on host; exp stays in fp32 range).
Output written as f16 [b, h, c, w] in DRAM (4KB DMA descriptors); host
transposes to [b, c, h, w] and widens to f32 in one parallel pass.
"""
import math
import numpy as np
from concurrent.futures import ThreadPoolExecutor
from contextlib import ExitStack

import concourse.bass as bass
import concourse.tile as tile
import concourse.mybir as mybir
from concourse import bacc, bass_utils
from concourse.masks import make_identity

F32 = mybir.dt.float32
F16 = mybir.dt.float16
AF = mybir.ActivationFunctionType
ALU = mybir.AluOpType

B_LOCAL = 2          # batches per core
CIN = 256
C = 128              # q/k/v channels
H = 128
W = 128
QKV = 3 * C          # 384
HB = 16              # h rows per x DMA chunk
SCALE = 1.0 / math.sqrt(128.0)

_POOL = ThreadPoolExecutor(max_workers=16)


def build(attn_dt=F16, proj_dt=F16, out_dt=F16):
    nc = bacc.Bacc(trn_type="TRN2", debug=False)
    x_d = nc.dram_tensor("x", [B_LOCAL, CIN, H, W], proj_dt, kind="ExternalInput").ap()
    w_d = nc.dram_tensor("w", [2, 128, QKV], proj_dt, kind="ExternalInput").ap()
    bqs_d = nc.dram_tensor("bqs", [128], F32, kind="ExternalInput").ap()
    bvs_d = nc.dram_tensor("bvs", [128], F32, kind="ExternalInput").ap()
    # out layout: [b, h, c, w]  (c,w contiguous -> 2KB descriptors per partition)
    o_d = nc.dram_tensor("o", [B_LOCAL, H, C, W], out_dt, kind="ExternalOutput").ap()

    with ExitStack() as ctx:
        tc = ctx.enter_context(tile.TileContext(nc))
        singles = ctx.enter_context(tc.tile_pool(name="singles", bufs=1))
        xpool = ctx.enter_context(tc.tile_pool(name="xp", bufs=2))
        exp_pool = ctx.enter_context(tc.tile_pool(name="expp", bufs=4))
        small = ctx.enter_context(tc.tile_pool(name="small", bufs=6))
        out8_pool = ctx.enter_context(tc.tile_pool(name="out8", bufs=2))
        ps_proj = ctx.enter_context(tc.tile_pool(name="psproj", bufs=2, space="PSUM"))
        ps_score = ctx.enter_context(tc.tile_pool(name="psscore", bufs=2, space="PSUM"))
        ps_vt = ctx.enter_context(tc.tile_pool(name="psvt", bufs=2, space="PSUM"))
        ps_out = ctx.enter_context(tc.tile_pool(name="psout", bufs=2, space="PSUM"))

        w_sb = singles.tile([128, 2, QKV], proj_dt)
        nc.sync.dma_start(out=w_sb, in_=w_d.rearrange("t p c -> p t c"))
        bqs_sb = singles.tile([128, 128], F32)
        nc.gpsimd.dma_start(
            out=bqs_sb,
            in_=bass.AP(tensor=bqs_d.tensor, offset=bqs_d.offset,
                        ap=[[0, 128], [1, 128]]),
        )
        bvs_sb = singles.tile([128, 128], F32)
        nc.gpsimd.dma_start(
            out=bvs_sb,
            in_=bass.AP(tensor=bvs_d.tensor, offset=bvs_d.offset,
                        ap=[[0, 128], [1, 128]]),
        )
        ones_sb = singles.tile([128, 1], attn_dt)
        nc.vector.memset(ones_sb, 1.0)
        ident = singles.tile([128, 128], attn_dt)
        make_identity(nc, ident)
        # qkvT[w, ch, h]: ch in [0,128)=q, [128,256)=k, [256,384)=v
        qkvT = singles.tile([128, QKV, H], attn_dt)
        # v1 slots: 8 channel-pairs in flight; ones column preset once
        NSLOT = 8
        v1_all = singles.tile([128, 2 * NSLOT, 132], attn_dt)
        nc.vector.memset(v1_all[:, :, 128], 1.0)

        for b in range(B_LOCAL):
            # ---------------- projection ----------------
            for hb in range(0, H, HB):
                x_t = xpool.tile([128, 2, HB, W], proj_dt)
                nc.sync.dma_start(
                    out=x_t,
                    in_=x_d[b, :, hb:hb + HB, :].rearrange(
                        "(t p) h w -> p t h w", p=128),
                )
                for hh in range(HB):
                    h = hb + hh
                    pp = ps_proj.tile([128, 512], F32)
                    nc.tensor.matmul(pp[:, 0:QKV], x_t[:, 0, hh, :], w_sb[:, 0, :],
                                     start=True, stop=False)
                    nc.tensor.matmul(pp[:, 0:QKV], x_t[:, 1, hh, :], w_sb[:, 1, :],
                                     start=False, stop=True)
                    if h % 2 == 0:
                        nc.scalar.copy(qkvT[:, :, h], pp[:, 0:QKV])
                    else:
                        nc.vector.tensor_copy(qkvT[:, :, h], pp[:, 0:QKV])

            # ---------------- attention ----------------
            for p in range(C // 2):
                c0 = 2 * p
                slot = p % NSLOT
                ss = ps_score.tile([128, 2, 132], F32)
                for cc in range(2):
                    c = c0 + cc
                    kT = qkvT[:, C + c, :]
                    nc.tensor.matmul(ss[:, cc, 128:129], kT, ones_sb,
                                     start=True, stop=True)
                    nc.tensor.matmul(ss[:, cc, 0:128], kT, qkvT[:, c, :],
                                     start=True, stop=True)
                bias2 = small.tile([128, 2], F32, tag="bias2")
                nc.vector.tensor_mul(bias2, ss[:, :, 128], bqs_sb[:, c0:c0 + 2])
                ex = exp_pool.tile([128, 2, 128], attn_dt)
                vt = ps_vt.tile([128, 2, 132], attn_dt)
                for cc in range(2):
                    c = c0 + cc
                    nc.scalar.activation(ex[:, cc, :], ss[:, cc, 0:128], AF.Exp,
                                         bias=bias2[:, cc:cc + 1], scale=SCALE)
                    nc.tensor.transpose(vt[:, cc, 0:128], qkvT[:, 2 * C + c, :],
                                        ident)
                nc.vector.tensor_copy(v1_all[:, 2 * slot:2 * slot + 2, 0:128],
                                      vt[:, :, 0:128])
                po = ps_out.tile([128, 2, 132], F32)
                for cc in range(2):
                    nc.tensor.matmul(po[:, cc, 0:129], ex[:, cc, :],
                                     v1_all[:, 2 * slot + cc, 0:129],
                                     start=True, stop=True)
                recip2 = small.tile([128, 2], F32, tag="recip2")
                nc.vector.reciprocal(recip2, po[:, :, 128])
                if p % 4 == 0:
                    o8 = out8_pool.tile([128, 8, 128], out_dt)
                for cc in range(2):
                    c = c0 + cc
                    if cc == 0:
                        nc.vector.tensor_scalar(
                            out=o8[:, c % 8, :], in0=po[:, cc, 0:128],
                            scalar1=recip2[:, cc:cc + 1], scalar2=bvs_sb[:, c:c + 1],
                            op0=ALU.mult, op1=ALU.add)
                    else:
                        nc.scalar.activation(
                            o8[:, c % 8, :], po[:, cc, 0:128], AF.Identity,
                            bias=bvs_sb[:, c:c + 1], scale=recip2[:, cc:cc + 1])
                if p % 4 == 3:
                    nc.sync.dma_start(out=o_d[b, :, c0 - 6:c0 + 2, :], in_=o8)
    nc.finalize()
    return nc


_CACHE = {}


def _get_nc():
    if "nc" not in _CACHE:
        _CACHE["nc"] = build()
    return _CACHE["nc"]


def _cast_f16(x):
    out = np.empty(x.shape, np.float16)
    def work(i):
        np.copyto(out[i], x[i], casting="same_kind")
    list(_POOL.map(work, range(x.shape[0])))
    return out


def prepare(x, wq, bq, wk, bk, wv, bv):
    x16 = _cast_f16(np.asarray(x))
    wq = np.asarray(wq, dtype=np.float32)
    wk = np.asarray(wk, dtype=np.float32)
    wv = np.asarray(wv, dtype=np.float32)
    bq = np.asarray(bq, dtype=np.float32)
    bv = np.asarray(bv, dtype=np.float32)

    wqkv = np.concatenate([wq, wk, wv], axis=0)              # [384, 256]
    wT = np.ascontiguousarray(wqkv.T.astype(np.float16)).reshape(2, 128, QKV)
    bqs = (bq * np.float32(SCALE)).astype(np.float32)

    nc = _get_nc()
    in_maps = [
        {"x": x16[2 * i:2 * i + 2], "w": wT, "bqs": bqs, "bvs": bv}
        for i in range(8)
    ]
    return in_maps, nc


def kernel(x, wq, bq, wk, bk, wv, bv, _trace=False):
    in_maps, nc = prepare(x, wq, bq, wk, bk, wv, bv)
    res = bass_utils.run_bass_kernel_spmd(
        nc, in_maps, core_ids=list(range(8)), trace=_trace)
    full = np.empty((16, C, H, W), np.float32)
    def work(i):
        o = np.asarray(res.results[i]["o"])          # [2, h, c, w] f16
        full[2 * i:2 * i + 2] = o.transpose(0, 2, 1, 3)
    list(_POOL.map(work, range(8)))
    if _trace:
        kernel._last_result = res
    return full
